# revision 9
# baseline (speedup 1.0000x reference)
"""Trainium2 Bass kernel for dense-gated MoE (nn_MoE_45947560132892).

Computation (per reference):
  logits = x @ gate_w + gate_b            [B, E]
  g      = renormalized top-2 softmax     [B, E]  (only top-2 nonzero)
  h      = gelu(x @ w1[e] + b1[e])        [B, E, H]
  out    = sum_e g[:, e] * (h[:, :, e] @ w2[e] + b2[e])   [B, D]

Shapes: B=16384, D=1024, H=256, E=8, K=2 (fp32).

Strategy: data-parallel over the 8 NeuronCores (2048 rows each), expert
weights replicated.  Per core, everything runs in a transposed dataflow:
x tiles are PE-transposed once into xT [D-part, B-free] (kept in both
fp32 and bf16).  The gating matmul runs in full fp32 — the top-2
selection must match the reference's routing decisions (0.2% of rows
have a 2nd/3rd-expert logit gap < 1e-3, so reduced precision here would
flip routes and blow up the error).  The expert matmuls run in bf16
(1 PE cycle/row, vs 4 for fp32; continuous ~4e-3 error).  The gating
scale is applied between gelu and mm2 via a selection-matrix broadcast
matmul; mm2 uses the scaled hT chunks as the stationary operand so its
output lands in natural [B, D] layout and DMAs straight out.
"""

import sys

import numpy as np

for _p in ("/opt/trn_rl_repo",):
    if _p not in sys.path:
        sys.path.insert(0, _p)

import ml_dtypes  # noqa: E402

import concourse.bass as bass  # noqa: E402
import concourse.bacc as bacc  # noqa: E402
import concourse.mybir as mybir  # noqa: E402
from concourse.bass_utils import run_bass_kernel_spmd  # noqa: E402
from concourse.tile import TileContext  # noqa: E402

# Problem constants (hardcoded per harness contract).
B_FULL, D, H, E = 16384, 1024, 8 * 32, 8
N_CORES = 8
B = B_FULL // N_CORES          # rows per core
CHUNK = 512                    # b-rows per pipeline chunk
N_CHUNKS = B // CHUNK
BT = CHUNK // 128              # 128-row tiles per chunk
KD = D // 128                  # k-tiles over D
H2 = H // 128                  # h-tiles over H
P = 128

F32 = mybir.dt.float32
BF16 = mybir.dt.bfloat16

# Set by test.py to capture a profile.
TRACE = False
LAST_RESULTS = None


def build_nc():
    nc = bacc.Bacc("TRN2", target_bir_lowering=False)

    x_d = nc.dram_tensor("x", [B, D], F32, kind="ExternalInput")
    gw_d = nc.dram_tensor("gate_w", [D, E], F32, kind="ExternalInput")
    gb_d = nc.dram_tensor("gate_b", [E, 1], F32, kind="ExternalInput")
    w1_d = nc.dram_tensor("w1bf", [E, D, H], BF16, kind="ExternalInput")
    b1_d = nc.dram_tensor("b1", [E, H], F32, kind="ExternalInput")
    w2_d = nc.dram_tensor("w2bf", [E, H, D], BF16, kind="ExternalInput")
    b2_d = nc.dram_tensor("b2bf", [E, D], BF16, kind="ExternalInput")
    id_d = nc.dram_tensor("ident", [P, P], F32, kind="ExternalInput")
    sel_d = nc.dram_tensor("sel", [E, E * P], BF16, kind="ExternalInput")
    out_d = nc.dram_tensor("out", [B, D], F32, kind="ExternalOutput")

    with TileContext(nc) as tc:
        with (
            tc.tile_pool(name="wpool", bufs=1) as wp,
            tc.tile_pool(name="xstage", bufs=3) as xsp,
            tc.tile_pool(name="xT", bufs=2) as xtp,
            tc.tile_pool(name="hg", bufs=3) as hgp,
            tc.tile_pool(name="hts", bufs=1) as htsp,
            tc.tile_pool(name="small", bufs=4) as smp,
            tc.tile_pool(name="gtpool", bufs=2) as gtp,
            tc.tile_pool(name="outp", bufs=2) as outp,
            tc.tile_pool(name="ps_tp", bufs=2, space="PSUM") as ps_tp,
            tc.tile_pool(name="ps_lg", bufs=1, space="PSUM") as ps_lg,
            tc.tile_pool(name="ps_gbc", bufs=1, space="PSUM") as ps_gbc,
            tc.tile_pool(name="ps_h", bufs=2, space="PSUM") as ps_h,
            tc.tile_pool(name="ps_o", bufs=1, space="PSUM") as ps_o,
        ):
            # ---- persistent weights / constants ----
            w1_sb = wp.tile([P, KD, E, H], BF16, tag="w1")
            w2_sb = wp.tile([P, H2, E, D], BF16, tag="w2")
            gw_sb = wp.tile([P, KD, E], F32, tag="gw")
            gb_sb = wp.tile([E, 1], F32, tag="gb")
            b1_sb = wp.tile([P, E, H2], F32, tag="b1")
            b2_sb = wp.tile([E, D], BF16, tag="b2")
            id_sb = wp.tile([P, P], F32, tag="id")
            sel_sb = wp.tile([E, E * P], BF16, tag="sel")

            nc.sync.dma_start(gw_sb, gw_d.rearrange("(k p) e -> p k e", p=P))
            nc.sync.dma_start(gb_sb, gb_d[:, :])
            nc.sync.dma_start(b1_sb, b1_d.rearrange("e (h2 p) -> p e h2", p=P))
            nc.sync.dma_start(b2_sb, b2_d[:, :])
            nc.sync.dma_start(id_sb, id_d[:, :])
            nc.sync.dma_start(sel_sb, sel_d[:, :])
            for e in range(E):
                nc.sync.dma_start(
                    w1_sb[:, :, e, :], w1_d[e].rearrange("(k p) h -> p k h", p=P)
                )
                nc.sync.dma_start(
                    w2_sb[:, :, e, :], w2_d[e].rearrange("(h2 p) d -> p h2 d", p=P)
                )

            for c in range(N_CHUNKS):
                b0 = c * CHUNK

                # ---- stage x, transpose into xT32 (fp32) + xTb (bf16) ----
                xT32 = xtp.tile([P, KD, CHUNK], F32, tag="xT32")
                xTb = xtp.tile([P, KD, CHUNK], BF16, tag="xTb")
                for bt in range(BT):
                    xs = xsp.tile([P, D], F32, tag="xs")
                    nc.sync.dma_start(xs, x_d[b0 + bt * P : b0 + (bt + 1) * P, :])
                    for k in range(KD):
                        tp = ps_tp.tile([P, P], F32, tag="tp")
                        nc.tensor.transpose(tp, xs[:, k * P : (k + 1) * P], id_sb)
                        nc.vector.tensor_copy(xT32[:, k, bt * P : (bt + 1) * P], tp)
                    # bf16 copy for the expert matmuls, from SBUF (keeps the
                    # PSUM transpose tiles single-reader for sync simplicity)
                    nc.scalar.copy(
                        xTb[:, :, bt * P : (bt + 1) * P],
                        xT32[:, :, bt * P : (bt + 1) * P],
                    )

                # ---- gating matmul (full fp32): logitsT [E, CHUNK] ----
                lg_ps = ps_lg.tile([P, CHUNK], F32, tag="lg")
                for k in range(KD):
                    nc.tensor.matmul(
                        lg_ps[:E, :],
                        gw_sb[:, k, :],
                        xT32[:, k, :],
                        start=(k == 0),
                        stop=(k == KD - 1),
                    )
                lgT_sb = gtp.tile([E, CHUNK], F32, tag="lgT")
                nc.scalar.activation(
                    lgT_sb, lg_ps[:E, :],
                    mybir.ActivationFunctionType.Identity, bias=gb_sb,
                )

                # ---- per 128-row tile: softmax + top-2 renorm weights ----
                gT_sb = gtp.tile([E, CHUNK], BF16, tag="gT")
                for bt in range(BT):
                    lt = ps_tp.tile([P, P], F32, tag="tp")
                    nc.tensor.transpose(
                        lt[:, :E], lgT_sb[:, bt * P : (bt + 1) * P], id_sb[:E, :E]
                    )
                    lg = smp.tile([P, E], F32, tag="lg")
                    nc.vector.tensor_copy(lg, lt[:, :E])

                    m8 = smp.tile([P, 8], F32, tag="m8")
                    nc.vector.max(out=m8, in_=lg)
                    nmax1 = smp.tile([P, 1], F32, tag="nmax1")
                    nc.vector.tensor_scalar_mul(nmax1, m8[:, 0:1], -1.0)
                    expl = smp.tile([P, E], F32, tag="expl")
                    nc.scalar.activation(
                        expl, lg, mybir.ActivationFunctionType.Exp, bias=nmax1
                    )
                    mask = smp.tile([P, E], F32, tag="mask")
                    nc.vector.tensor_scalar(
                        mask, lg, m8[:, 1:2], None, mybir.AluOpType.is_ge
                    )
                    num = smp.tile([P, E], F32, tag="num")
                    nc.vector.tensor_tensor(num, expl, mask, mybir.AluOpType.mult)
                    den = smp.tile([P, 1], F32, tag="den")
                    nc.vector.tensor_reduce(
                        den, num, axis=mybir.AxisListType.X, op=mybir.AluOpType.add
                    )
                    rden = smp.tile([P, 1], F32, tag="rden")
                    nc.vector.reciprocal(rden, den)
                    g = smp.tile([P, E], F32, tag="g")
                    nc.vector.tensor_scalar_mul(g, num, rden)

                    gt = ps_tp.tile([P, P], F32, tag="tp")
                    nc.tensor.transpose(gt[:E, :], g, id_sb)
                    nc.vector.tensor_copy(gT_sb[:, bt * P : (bt + 1) * P], gt[:E, :])

                # ---- experts: mm1 (bf16) + gelu + gating scale -> hTs bf16 ----
                hTs = htsp.tile([P, E, H2, CHUNK], BF16, tag="hTs")
                for e in range(E):
                    gbc = ps_gbc.tile([P, CHUNK], F32, tag="gbc")
                    nc.tensor.matmul(
                        gbc, sel_sb[:, e * P : (e + 1) * P], gT_sb,
                        start=True, stop=True,
                    )
                    for h2 in range(H2):
                        hp = ps_h.tile([P, CHUNK], F32, tag="h")
                        for k in range(KD):
                            nc.tensor.matmul(
                                hp,
                                w1_sb[:, k, e, h2 * P : (h2 + 1) * P],
                                xTb[:, k, :],
                                start=(k == 0),
                                stop=(k == KD - 1),
                            )
                        hg = hgp.tile([P, CHUNK], BF16, tag="hg")
                        nc.scalar.activation(
                            hg, hp, mybir.ActivationFunctionType.Gelu,
                            bias=b1_sb[:, e, h2 : h2 + 1],
                        )
                        nc.vector.tensor_tensor(
                            hTs[:, e, h2, :], hg, gbc, mybir.AluOpType.mult
                        )

                # ---- mm2 (bf16) + b2: out tiles in natural [b, d] layout ----
                for bt in range(BT):
                    op = ps_o.tile([P, D], F32, tag="o")
                    for dh in range(2):
                        nc.tensor.matmul(
                            op[:, dh * 512 : (dh + 1) * 512],
                            gT_sb[:, bt * P : (bt + 1) * P],
                            b2_sb[:, dh * 512 : (dh + 1) * 512],
                            start=True,
                            stop=False,
                        )
                    for e in range(E):
                        for h2 in range(H2):
                            lhsT = hTs[:, e, h2, bt * P : (bt + 1) * P]
                            for dh in range(2):
                                nc.tensor.matmul(
                                    op[:, dh * 512 : (dh + 1) * 512],
                                    lhsT,
                                    w2_sb[:, h2, e, dh * 512 : (dh + 1) * 512],
                                    start=False,
                                    stop=(e == E - 1 and h2 == H2 - 1),
                                )
                    ob = outp.tile([P, D], F32, tag="out")
                    nc.vector.tensor_copy(ob, op)
                    nc.sync.dma_start(out_d[b0 + bt * P : b0 + (bt + 1) * P, :], ob)

    nc.compile()
    return nc


_NC_CACHE = None


def kernel(**inputs) -> np.ndarray:
    global LAST_RESULTS, _NC_CACHE
    x = np.ascontiguousarray(np.asarray(inputs["x"], dtype=np.float32))
    gate_w = np.ascontiguousarray(np.asarray(inputs["gate_w"], dtype=np.float32))
    gate_b = np.ascontiguousarray(
        np.asarray(inputs["gate_b"], dtype=np.float32).reshape(E, 1)
    )
    w1 = np.asarray(inputs["w1"], dtype=np.float32)
    b1 = np.ascontiguousarray(np.asarray(inputs["b1"], dtype=np.float32))
    w2 = np.asarray(inputs["w2"], dtype=np.float32)
    b2 = np.asarray(inputs["b2"], dtype=np.float32)

    w1bf = np.ascontiguousarray(w1.astype(ml_dtypes.bfloat16))
    w2bf = np.ascontiguousarray(w2.astype(ml_dtypes.bfloat16))
    b2bf = np.ascontiguousarray(b2.astype(ml_dtypes.bfloat16))
    ident = np.eye(P, dtype=np.float32)
    sel = np.zeros((E, E * P), dtype=ml_dtypes.bfloat16)
    for e in range(E):
        sel[e, e * P : (e + 1) * P] = 1.0

    if _NC_CACHE is None:
        _NC_CACHE = build_nc()
    nc = _NC_CACHE

    shared = {
        "gate_w": gate_w,
        "gate_b": gate_b,
        "w1bf": w1bf,
        "b1": b1,
        "w2bf": w2bf,
        "b2bf": b2bf,
        "ident": ident,
        "sel": sel,
    }
    in_maps = [
        {"x": np.ascontiguousarray(x[i * B : (i + 1) * B]), **shared}
        for i in range(N_CORES)
    ]

    res = run_bass_kernel_spmd(
        nc, in_maps, core_ids=list(range(N_CORES)), trace=TRACE
    )
    LAST_RESULTS = res
    out = np.concatenate([r["out"] for r in res.results], axis=0)
    return out.astype(np.float32)
